# revision 1
# baseline (speedup 1.0000x reference)
"""Trainium2 Bass kernel for nn_AttentionBlock (dense transformer attention
with L2-normalized projections, qk L2-norm + scale, interleaved RoPE, group
mask, softmax, output projection).

Sharding: 8 cores = 2 batches x 4 head-groups (4 heads of d=64 each).
Each core computes its batch/head-group's attention and a partial output
projection over its 256 channels; host sums the 4 partials per batch.

Device dataflow (per core):
  P1: qkT = Wqk_norm.T @ xT as stacked [q_h(64); k_h(64)] tiles; L2-norm
      over d via ones-matmul partition reduction; scale by s; RoPE via
      pair-rotation matmul; DMA-split into base-0 qh/kh tiles.
  P2a: v = x @ Wv_norm.T (natural layout, bf16), x re-streamed from DRAM.
  P2b: Wo row norms via ones-matmul over Wo.T squares; normalized wos.
  P3: per (i-chunk, head, j-tile): S^T = kh.T qh (PSUM); P = exp(8*S^T)
      bf16; PM = (mask_row == mask_col) * P in one fused DVE op;
      outT += v.T @ PM (PSUM accum over j); Z += ones.T @ PM;
      evict outT with 1/Z multiply fused.
  P5: final = (out/Z).T @ nWo_cols -> natural [n, 1024] partial -> DRAM.
"""

import functools
import sys
from contextlib import ExitStack

import numpy as np

sys.path.insert(0, "/opt/trn_rl_repo")

import ml_dtypes

# ---- problem constants (hardcoded per contract) ----
B = 2
N_FULL = 2048
DIM = 1024
HEADS = 16
DH = 64
N_CORES = 8
GROUPS = 4  # head groups (cores per batch)
NH = HEADS // GROUPS  # heads per core = 4
ROPE_THETA = 10000.0


def _build_program(N=N_FULL, DIN=DIM, NH_=NH, mm_big="float32r"):
    """Build the single-core SPMD Bass program. Identical on all cores."""
    import os
    _PH = int(os.environ.get("KPHASES", "7"))  # bit0: P3 attn, bit1: P5 final
    import concourse.bass as bass
    import concourse.mybir as mybir
    import concourse.tile as tile
    from concourse import bacc

    f32 = mybir.dt.float32
    bf16 = mybir.dt.bfloat16
    f32r = getattr(mybir.dt, mm_big)

    C = NH_ * DH            # per-core q (or k, or v) channels
    CT = C // 128           # channel tiles (2 for full size)
    KT = DIN // 128         # contraction tiles for projections
    NT = N // 128           # sequence tiles
    PCH = min(512, N)       # free-dim chunk for f32r matmuls
    NCH = N // PCH
    IW = min(512, N)        # i-chunk width in attention
    NIH = N // IW
    IWC = IW // PCH         # 512-chunks per i-chunk
    FCH = min(512, DIN)     # final/wnorm free chunk
    NFC = DIN // FCH
    assert DH == 64 and C % 128 == 0 and DIN % 128 == 0 and N % 128 == 0

    nc = bacc.Bacc(None, target_bir_lowering=False)

    dt_in = lambda name, shape, dt=f32: nc.dram_tensor(
        name, list(shape), dt, kind="ExternalInput"
    )
    xT_d = dt_in("xt", (DIN, N), bf16)
    wqk_d = dt_in("wqk", (DIN, NH_ * 128), bf16)
    wv_d = dt_in("wv", (DIN, C), bf16)
    wos_d = dt_in("wos", (C, DIN))
    wofT_d = dt_in("woft", (DIN, DIN), bf16)
    cs_d = dt_in("cs", (128, N), bf16)
    sn_d = dt_in("sn", (128, N), bf16)
    scol_d = dt_in("scol", (128, NH_))
    mrow_d = dt_in("mrow", (1, N), bf16)
    mcol_d = dt_in("mcol", (128, NT), bf16)
    mcolf_d = dt_in("mcolf", (128, NT))
    rmat_d = dt_in("rmat", (128, 128), bf16)
    ones2_d = dt_in("ones2", (128, 2))
    ones2T_d = dt_in("ones2t", (2, 128))
    ones1f_d = dt_in("ones1f", (128, 1))
    ones1b_d = dt_in("ones1b", (128, 1), bf16)
    out_d = nc.dram_tensor("out", [N, DIN], f32, kind="ExternalOutput")

    AF = mybir.ActivationFunctionType
    OP = mybir.AluOpType

    def r(ap):  # matmul-input tiles/tensors already carry the f32r dtype
        return ap

    with nc.allow_low_precision(reason="float32r tags for PE-speed matmuls"), \
         tile.TileContext(nc) as tc:
        with ExitStack() as es:
            constp = es.enter_context(tc.tile_pool(name="const", bufs=1))
            qkp = es.enter_context(tc.tile_pool(name="qk", bufs=1))
            # small constants
            scol_t = constp.tile([128, NH_], f32, name="scol", tag="scol")
            mrow_t = constp.tile([1, N], bf16, name="mrow", tag="mrow")
            mcol_t = constp.tile([128, NT], bf16, name="mcol", tag="mcol")
            mcolf_t = constp.tile([128, NT], f32, name="mcolf", tag="mcolf")
            rmat_t = constp.tile([128, 128], bf16, name="rmat", tag="rmat")
            ones2_t = constp.tile([128, 2], f32, name="ones2", tag="ones2")
            ones2T_t = constp.tile([2, 128], f32, name="ones2t", tag="ones2t")
            ones1f_t = constp.tile([128, 1], f32, name="ones1f", tag="ones1f")
            ones1b_t = constp.tile([128, 1], bf16, name="ones1b", tag="ones1b")
            for t_, d_ in (
                (scol_t, scol_d), (mrow_t, mrow_d), (mcol_t, mcol_d),
                (mcolf_t, mcolf_d),
                (rmat_t, rmat_d), (ones2_t, ones2_d), (ones2T_t, ones2T_d),
                (ones1f_t, ones1f_d), (ones1b_t, ones1b_d),
            ):
                nc.sync.dma_start(out=t_[:], in_=d_[:])

            qh = [qkp.tile([64, N], bf16, name=f"qh{t}", tag=f"qh{t}")
                  for t in range(NH_)]
            kh = [qkp.tile([64, N], bf16, name=f"kh{t}", tag=f"kh{t}")
                  for t in range(NH_)]

            vnp = es.enter_context(tc.tile_pool(name="vnp", bufs=1))
            vn = [[vnp.tile([128, DH + 1], bf16, name=f"vn{j}_{t}",
                            tag=f"vn{j}_{t}") for t in range(NH_)]
                  for j in range(NT)]

            # ---------- P1: qk projection + l2norm + scale + rope ----------
            with (
                tc.tile_pool(name="xtp", bufs=1) as xtp,
                tc.tile_pool(name="wqkp", bufs=2) as wqkp,
                tc.tile_pool(name="qkrp", bufs=2) as qkrp,
                tc.tile_pool(name="cschp", bufs=1) as cschp,
                tc.tile_pool(name="p1sb", bufs=4) as p1sb,
                tc.tile_pool(name="wbd", bufs=2, space="DRAM") as wbd,
            ):
                cs_t = cschp.tile([128, N], bf16, name="cs", tag="cs")
                sn_t = cschp.tile([128, N], bf16, name="sn", tag="sn")
                nc.sync.dma_start(out=cs_t[:], in_=cs_d[:])
                nc.sync.dma_start(out=sn_t[:], in_=sn_d[:])
                xt = [xtp.tile([128, N], bf16, name=f"xt{k}", tag=f"xt{k}")
                      for k in range(KT)]
                for k in range(KT):
                    nc.sync.dma_start(out=xt[k][:], in_=xT_d[128 * k:128 * (k + 1), :])

                with (
                    tc.tile_pool(name="ps_qk", bufs=2, space="PSUM") as ps_qk,
                    tc.tile_pool(name="ps_n2", bufs=2, space="PSUM") as ps_n2,
                    tc.tile_pool(name="ps_rb", bufs=1, space="PSUM") as ps_rb,
                    tc.tile_pool(name="ps_rot", bufs=2, space="PSUM") as ps_rot,
                    tc.tile_pool(name="ps_nw", bufs=1, space="PSUM") as ps_nw,
                ):
                  for t in range(NH_):
                    wqkt = [wqkp.tile([128, 128], bf16, name=f"wqkt{k}", tag=f"wqkt{k}")
                            for k in range(KT)]
                    for k in range(KT):
                        nc.sync.dma_start(
                            out=wqkt[k][:],
                            in_=wqk_d[128 * k:128 * (k + 1), 128 * t:128 * (t + 1)],
                        )
                    # W row norms for this head's 128 rows (over all KT k-tiles)
                    nw_ps = ps_nw.tile([1, 128], f32, name="nwps", tag="nwps")
                    for k in range(KT):
                        wsq2 = p1sb.tile([128, 128], f32, name="wsq2", tag="wsq2")
                        nc.scalar.square(wsq2[:], wqkt[k][:])
                        nc.tensor.matmul(nw_ps[:], r(ones1f_t[:]), r(wsq2[:]),
                                         start=(k == 0), stop=(k == KT - 1))
                    wn = p1sb.tile([1, 128], f32, name="wn", tag="wn")
                    nc.scalar.sqrt(wn[:], nw_ps[:])
                    nc.vector.reciprocal(wn[:], wn[:])
                    wnd = wbd.tile([1, 128], f32, name="wnd", tag="wnd")
                    nc.sync.dma_start(out=wnd[:], in_=wn[:])
                    wncol = p1sb.tile([128, 1], f32, name="wncol", tag="wncol")
                    nc.sync.dma_start(out=wncol[:],
                                      in_=wnd[:].rearrange("a b -> b a"))
                    qkr = qkrp.tile([128, N], bf16, name="qkr", tag="qkr")
                    for ch in range(NCH):
                        sl = slice(PCH * ch, PCH * (ch + 1))
                        csc = cs_t[:, sl]
                        snc = sn_t[:, sl]
                        qk_ps = ps_qk.tile([128, PCH], f32, name="qkps", tag="qkps")
                        for k in range(KT):
                            nc.tensor.matmul(
                                qk_ps[:], r(wqkt[k][:]), r(xt[k][:, sl]),
                                start=(k == 0), stop=(k == KT - 1),
                            )
                        qraw = p1sb.tile([128, PCH], f32, name="qraw", tag="qraw")
                        nc.scalar.mul(qraw[:], qk_ps[:], wncol[:])
                        qsq = p1sb.tile([128, PCH], f32, name="qsq", tag="qsq")
                        nc.vector.tensor_mul(qsq[:], qraw[:], qraw[:])
                        # per-half sum of squares (q rows 0-63, k rows 64-127)
                        n2_ps = ps_n2.tile([2, PCH], f32, name="n2ps", tag="n2ps")
                        nc.tensor.matmul(n2_ps[:], r(ones2_t[:]), r(qsq[:]),
                                         start=True, stop=True)
                        nrm = p1sb.tile([2, PCH], f32, name="nrm", tag="nrm")
                        nc.scalar.sqrt(nrm[:], n2_ps[:])
                        nc.vector.reciprocal(nrm[:], nrm[:])
                        rb_ps = ps_rb.tile([128, PCH], f32, name="rbps", tag="rbps")
                        nc.tensor.matmul(rb_ps[:], r(ones2T_t[:]), r(nrm[:]),
                                         start=True, stop=True)
                        qn = p1sb.tile([128, PCH], bf16, name="qn", tag="qn")
                        nc.vector.scalar_tensor_tensor(
                            qn[:], qraw[:], scol_t[:, t:t + 1], rb_ps[:],
                            OP.mult, OP.mult,
                        )
                        rot_ps = ps_rot.tile([128, PCH], f32, name="rotps", tag="rotps")
                        nc.tensor.matmul(rot_ps[:], r(rmat_t[:]), r(qn[:]),
                                         start=True, stop=True)
                        nc.vector.tensor_mul(qkr[:, sl], qn[:], csc)
                        tb = p1sb.tile([128, PCH], f32, name="tb", tag="tb")
                        nc.vector.tensor_mul(tb[:], rot_ps[:], snc)
                        nc.vector.tensor_add(qkr[:, sl], qkr[:, sl], tb[:])
                    # split stacked tile into base-0 q/k tiles
                    nc.sync.dma_start(out=qh[t][:], in_=qkr[0:64, :])
                    nc.sync.dma_start(out=kh[t][:], in_=qkr[64:128, :])

                # -- P2a: v projection (resident x) --
                with (
                    tc.tile_pool(name="wvp", bufs=1) as wvp,
                    tc.tile_pool(name="p2a", bufs=2) as p2a,
                    tc.tile_pool(name="vbd", bufs=1, space="DRAM") as vbd,
                    tc.tile_pool(name="ps_v", bufs=2, space="PSUM") as ps_v,
                    tc.tile_pool(name="ps_nv", bufs=1, space="PSUM") as ps_nv,
                ):
                    wv = [wvp.tile([128, C], bf16, name=f"wv{k}", tag=f"wv{k}")
                          for k in range(KT)]
                    for k in range(KT):
                        nc.sync.dma_start(out=wv[k][:], in_=wv_d[128 * k:128 * (k + 1), :])
                    nv_ps = ps_nv.tile([1, C], f32, name="nvps", tag="nvps")
                    for k in range(KT):
                        wvsq = p2a.tile([128, C], f32, name="wvsq", tag="wvsq")
                        nc.scalar.square(wvsq[:], wv[k][:])
                        nc.tensor.matmul(nv_ps[:], r(ones1f_t[:]), r(wvsq[:]),
                                         start=(k == 0), stop=(k == KT - 1))
                    nv = p2a.tile([1, C], f32, name="nv", tag="nv")
                    nc.scalar.sqrt(nv[:], nv_ps[:])
                    nc.vector.reciprocal(nv[:], nv[:])
                    nv_d = vbd.tile([1, C], f32, name="nvd", tag="nvd")
                    nc.sync.dma_start(out=nv_d[:], in_=nv[:])
                    rnovb = p2a.tile([128, C], f32, name="rnovb", tag="rnovb")
                    nc.sync.dma_start(out=rnovb[:], in_=nv_d[:].to_broadcast([128, C]))
                    for j in range(NT):
                        v_ps = ps_v.tile([128, C], f32, name="vps", tag="vps")
                        for k in range(KT):
                            nc.tensor.matmul(v_ps[:],
                                             r(xt[k][:, 128 * j:128 * (j + 1)]),
                                             r(wv[k][:]),
                                             start=(k == 0), stop=(k == KT - 1))
                        for t in range(NH_):
                            sl64 = slice(64 * t, 64 * (t + 1))
                            nc.vector.tensor_mul(vn[j][t][:, 0:DH], v_ps[:, sl64],
                                                 rnovb[:, sl64])
                            nc.vector.memset(vn[j][t][:, DH:DH + 1], 1.0)


            # ---------- P2b: Wo row norms + normalized wos ----------
            nwosp = es.enter_context(tc.tile_pool(name="nwosp", bufs=1))
            nwos = [nwosp.tile([64, DIN], bf16, name=f"nwos{c}", tag=f"nwos{c}")
                    for c in range(NH_)]
            with (
                tc.tile_pool(name="wofp", bufs=2) as wofp,
                tc.tile_pool(name="p2sb", bufs=2) as p2sb,
                tc.tile_pool(name="drb", bufs=2, space="DRAM") as drb,
                tc.tile_pool(name="ps_wn", bufs=1, space="PSUM") as ps_wn,
            ):
                wn_ps = ps_wn.tile([1, DIN], f32, name="wnps", tag="wnps")
                for k in range(KT):
                    woft = wofp.tile([128, DIN], bf16, name="woft", tag="woft")
                    nc.sync.dma_start(out=woft[:], in_=wofT_d[128 * k:128 * (k + 1), :])
                    wsq = p2sb.tile([128, DIN], f32, name="wsq", tag="wsq")
                    nc.scalar.square(wsq[:], woft[:])
                    for ch in range(NFC):
                        sl = slice(FCH * ch, FCH * (ch + 1))
                        nc.tensor.matmul(wn_ps[:, sl], r(ones1f_t[:]), r(wsq[:, sl]),
                                         start=(k == 0), stop=(k == KT - 1))
                rno = p2sb.tile([1, DIN], f32, name="rno", tag="rno")
                nc.scalar.sqrt(rno[:], wn_ps[:])
                nc.vector.tensor_scalar_max(rno[:], rno[:], 1e-12)
                nc.vector.reciprocal(rno[:], rno[:])
                rno_d = drb.tile([1, DIN], f32, name="rnod", tag="rnod")
                nc.sync.dma_start(out=rno_d[:], in_=rno[:])
                rnob = p2sb.tile([128, DIN], f32, name="rnob", tag="rnob")
                nc.sync.dma_start(out=rnob[:], in_=rno_d[:].to_broadcast([128, DIN]))
                for c in range(NH_):
                    wos = p2sb.tile([64, DIN], f32, name="wos", tag="wos")
                    nc.sync.dma_start(out=wos[:], in_=wos_d[64 * c:64 * (c + 1), :])
                    nc.vector.tensor_mul(nwos[c][:], wos[:], rnob[0:64, :])

            # ---------- P3: attention ----------
            outcp = es.enter_context(tc.tile_pool(name="outcp", bufs=1))
            outc = [outcp.tile([64, N], bf16, name=f"outc{c}", tag=f"outc{c}")
                    for c in range(NH_)]
            with (
                tc.tile_pool(name="mbp", bufs=1) as mbp,
                tc.tile_pool(name="pp", bufs=5) as pp,
                tc.tile_pool(name="pmp", bufs=5) as pmp,
                tc.tile_pool(name="ztp", bufs=3) as ztp,
                tc.tile_pool(name="rzbp", bufs=3) as rzbp,
                tc.tile_pool(name="zdr", bufs=2, space="DRAM") as zdr,
                tc.tile_pool(name="ps_s", bufs=4, space="PSUM") as ps_s,
                tc.tile_pool(name="ps_av", bufs=2, space="PSUM") as ps_av,
            ):
                mrowb_t = mbp.tile([128, N], bf16, name="mrowb", tag="mrowb")
                nc.sync.dma_start(out=mrowb_t[:],
                                  in_=mrow_d[:].to_broadcast([128, N]))
                Ms = [mbp.tile([128, IW], bf16, name=f"mm{j}", tag=f"mm{j}")
                      for j in range(NT)]
                for ih in range(NIH if _PH & 1 else 0):
                    isl = slice(IW * ih, IW * (ih + 1))
                    for t in range(NH_):
                        av_ps = ps_av.tile([DH + 1, IW], f32, name="avps", tag="avps")
                        for j in range(NT):
                            s_ps = ps_s.tile([128, IW], f32, name="sps", tag="sps")
                            for c2 in range(IWC):
                                csl = slice(PCH * c2, PCH * (c2 + 1))
                                gsl = slice(IW * ih + PCH * c2,
                                            IW * ih + PCH * (c2 + 1))
                                nc.tensor.matmul(
                                    s_ps[:, csl],
                                    r(kh[t][:, 128 * j:128 * (j + 1)]),
                                    r(qh[t][:, gsl]),
                                    start=True, stop=True,
                                )
                            p_t = pp.tile([128, IW], bf16, name="p", tag="p")
                            nc.scalar.activation(p_t[:], s_ps[:], AF.Exp, scale=8.0)
                            if t == 0:
                                nc.vector.tensor_scalar(
                                    Ms[j][:], mrowb_t[:, isl],
                                    mcolf_t[:, j:j + 1], None, OP.is_equal,
                                )
                            pm_t = pmp.tile([128, IW], bf16, name="pm", tag="pm")
                            nc.vector.tensor_mul(pm_t[:], p_t[:], Ms[j][:])
                            for c2 in range(IWC):
                                csl = slice(PCH * c2, PCH * (c2 + 1))
                                nc.tensor.matmul(
                                    av_ps[:, csl],
                                    vn[j][t][:],
                                    pm_t[:, csl],
                                    start=(j == 0), stop=(j == NT - 1),
                                )
                        zt = ztp.tile([DH + 1, IW], f32, name="zt", tag="zt")
                        nc.scalar.copy(zt[DH:DH + 1, :], av_ps[DH:DH + 1, :])
                        nc.vector.reciprocal(zt[DH:DH + 1, :], zt[DH:DH + 1, :])
                        zt_d = zdr.tile([1, IW], f32, name="ztd", tag="ztd")
                        nc.gpsimd.dma_start(out=zt_d[:], in_=zt[DH:DH + 1, :])
                        rzb = rzbp.tile([64, IW], f32, name="rzb", tag="rzb")
                        nc.gpsimd.dma_start(out=rzb[:],
                                            in_=zt_d[:].to_broadcast([64, IW]))
                        nc.vector.tensor_mul(outc[t][:, isl], av_ps[0:DH, :],
                                             rzb[:])

            # ---------- P5: final projection ----------
            with (
                tc.tile_pool(name="foutp", bufs=3) as foutp,
                tc.tile_pool(name="ps_f", bufs=2, space="PSUM") as ps_f,
            ):
                if not (_PH & 1):
                    for t in range(NH_):
                        nc.vector.memset(outc[t][:], 0.0)
                for it in range(NT if _PH & 2 else 0):
                    f_ps = ps_f.tile([128, DIN], f32, name="fps", tag="fps")
                    for c in range(NH_):
                        for ch in range(NFC):
                            sl = slice(FCH * ch, FCH * (ch + 1))
                            nc.tensor.matmul(
                                f_ps[:, sl],
                                r(outc[c][:, 128 * it:128 * (it + 1)]),
                                r(nwos[c][:, sl]),
                                start=(c == 0), stop=(c == NH_ - 1),
                            )
                    fo = foutp.tile([128, DIN], f32, name="fo", tag="fo")
                    nc.scalar.copy(fo[:], f_ps[:])
                    nc.gpsimd.dma_start(out=out_d[128 * it:128 * (it + 1), :],
                                        in_=fo[:])
                if not (_PH & 2):
                    fo = foutp.tile([128, DIN], f32, name="fo", tag="fo")
                    nc.vector.memset(fo[:], 0.0)
                    for it in range(NT):
                        nc.gpsimd.dma_start(
                            out=out_d[128 * it:128 * (it + 1), :], in_=fo[:])

    nc.compile()
    return nc


def _prep_core_inputs(x_b, mask_b, Wq, Wk, Wv, Wo, qk_scale, g4,
                      N=N_FULL, DIN=DIM, NH_=NH):
    """Host-side numpy prep of one core's input map."""
    f32 = np.float32
    bf = ml_dtypes.bfloat16
    C = NH_ * DH
    NT = N // 128
    rows = slice(C * g4, C * (g4 + 1))

    wqk = np.empty((DIN, NH_ * 128), f32)
    for t in range(NH_):
        gh = NH_ * g4 + t
        wqk[:, 128 * t:128 * t + 64] = Wq[64 * gh:64 * gh + 64, :].T
        wqk[:, 128 * t + 64:128 * t + 128] = Wk[64 * gh:64 * gh + 64, :].T
    wv = np.ascontiguousarray(Wv[rows, :].T, f32)
    wos = np.ascontiguousarray(Wo[:, rows].T, f32)
    woft = np.ascontiguousarray(Wo.T, f32)

    inv_freq = 1.0 / (ROPE_THETA ** (np.arange(0, DH, 2, dtype=f32) / DH))
    ang = np.arange(N, dtype=f32)[:, None] * inv_freq[None, :]
    fr = np.repeat(ang, 2, axis=-1)  # (N, DH)
    cs64 = np.cos(fr).T  # (DH, N)
    sn64 = np.sin(fr).T
    cs = np.concatenate([cs64, cs64], 0).astype(f32)
    sn = np.concatenate([sn64, sn64], 0).astype(f32)

    s = (qk_scale * np.sqrt(DIN)).astype(f32)  # (DIN,)
    scol = np.empty((128, NH_), f32)
    for t in range(NH_):
        gh = NH_ * g4 + t
        sh = s[64 * gh:64 * gh + 64]
        scol[0:64, t] = sh
        scol[64:128, t] = sh

    mrow = mask_b.astype(f32).reshape(1, N).astype(bf)
    mcol = np.ascontiguousarray(mask_b.astype(f32).reshape(NT, 128).T).astype(bf)

    rmat = np.zeros((128, 128), f32)
    for p in range(0, 128, 2):
        rmat[p + 1, p] = -1.0
        rmat[p, p + 1] = 1.0

    ones2 = np.zeros((128, 2), f32)
    ones2[0:64, 0] = 1.0
    ones2[64:128, 1] = 1.0

    return {
        "xt": np.ascontiguousarray(x_b.T).astype(bf),
        "wqk": wqk.astype(bf), "wv": wv.astype(bf), "wos": wos,
        "woft": woft.astype(bf),
        "cs": cs.astype(bf), "sn": sn.astype(bf), "scol": scol,
        "mrow": mrow, "mcol": mcol, "mcolf": mcol.astype(f32),
        "rmat": rmat.astype(bf),
        "ones2": ones2, "ones2t": np.ascontiguousarray(ones2.T),
        "ones1f": np.ones((128, 1), f32),
        "ones1b": np.ones((128, 1), bf),
    }


@functools.lru_cache(maxsize=1)
def _get_program():
    return _build_program()


def kernel(x, mask, Wq, Wk, Wv, Wo, qk_scale_param, _trace=False):
    from concourse.bass_utils import run_bass_kernel_spmd

    x = np.asarray(x, np.float32)
    mask = np.asarray(mask)
    Wq = np.asarray(Wq, np.float32)
    Wk = np.asarray(Wk, np.float32)
    Wv = np.asarray(Wv, np.float32)
    Wo = np.asarray(Wo, np.float32)
    qs = np.asarray(qk_scale_param, np.float32)

    nc = _get_program()
    in_maps = []
    for core in range(N_CORES):
        b, g4 = core // GROUPS, core % GROUPS
        in_maps.append(_prep_core_inputs(x[b], mask[b], Wq, Wk, Wv, Wo, qs, g4))

    res = run_bass_kernel_spmd(nc, in_maps, core_ids=list(range(N_CORES)),
                               trace=_trace)
    outs = [res.results[c]["out"] for c in range(N_CORES)]
    full = np.stack([
        outs[0] + outs[1] + outs[2] + outs[3],
        outs[4] + outs[5] + outs[6] + outs[7],
    ]).astype(np.float32)
    if _trace:
        kernel.last_results = res
    return full

